# revision 2
# baseline (speedup 1.0000x reference)
"""Trainium2 Bass kernel for nn_CusparseDynamicLinear:
    out = data @ (mask * weight)^T + bias
  data  [8192, 4096] fp32
  weight[4096, 4096] fp32
  mask  [4096, 4096] fp32 (0/1)
  bias  [4096]       fp32
  out   [8192, 4096] fp32

Strategy: 8 NeuronCores, sharded 2-way on tokens (M) x 4-way on
out_features (N). Host does pure layout marshalling (transpose to
K-major for the PE's contraction-on-partitions requirement); all
FLOPs (mask multiply, matmul, bias add) run on device.

Per core:
  dT [K=4096, M_c=4096]  (transposed data shard, fp32)
  wT/mT [K=4096, N_c=1024] (transposed weight/mask shards)
  bias_c [1024]
  out_c [M_c, N_c]

Device program: mask*weight on DVE producing a resident fp32r tile
(K on partitions); data tiles DMA-cast fp32->fp32r in flight (SWDGE);
fp32r matmuls accumulate K in PSUM; DVE adds bias from PSUM; DMA out.
fp32r streams at bf16 speed for free-dim>=256 with ~16x better
accuracy (measured absmax rel err ~2e-4 vs 2.6e-3 for bf16).
"""
import os
import numpy as np

import concourse.bass as bass
import concourse.tile as tile
import concourse.mybir as mybir
from concourse import bacc
from concourse.bass_utils import run_bass_kernel_spmd

M, N, K = 8192, 4096, 4096
M_SH, N_SH = 2, 4
M_C, N_C = M // M_SH, N // N_SH   # 4096, 1024
P = 128
KS = K // P                       # 32 k-subtiles
N_TILE = 512                      # psum free dim (one bank of fp32)
MG = 128                          # m-tile (stationary free dim)

_NC_CACHE = None
LAST_RESULTS = None  # BassKernelResults of the most recent run (for test.py)


def _build_nc():
    nc = bacc.Bacc()
    f32 = mybir.dt.float32
    f32r = mybir.dt.float32r

    dT = nc.dram_tensor("dT", [K, M_C], f32, kind="ExternalInput")
    wT = nc.dram_tensor("wT", [K, N_C], f32, kind="ExternalInput")
    mT = nc.dram_tensor("mT", [K, N_C], f32, kind="ExternalInput")
    bias = nc.dram_tensor("bias", [N_C], f32, kind="ExternalInput")
    out = nc.dram_tensor("out", [M_C, N_C], f32, kind="ExternalOutput")

    dT3 = dT.rearrange("(ks p) m -> p ks m", p=P)
    wT3 = wT.rearrange("(ks p) n -> p ks n", p=P)
    mT3 = mT.rearrange("(ks p) n -> p ks n", p=P)
    out3 = out.rearrange("(mt p) n -> p mt n", p=P)

    with tile.TileContext(nc) as tc:
        with (
            tc.tile_pool(name="wres", bufs=1) as wres,
            tc.tile_pool(name="wstage", bufs=2) as wstage,
            tc.tile_pool(name="dpool", bufs=2) as dpool,
            tc.tile_pool(name="opool", bufs=4) as opool,
            tc.tile_pool(name="pspool", bufs=4, space="PSUM") as pspool,
        ):
            # --- phase 1: masked weights, fp32r, resident [P, KS, N_C] ---
            wm = wres.tile([P, KS, N_C], f32r, tag="wm")
            for k in range(KS):
                ws = wstage.tile([P, N_C], f32, tag="ws")
                ms = wstage.tile([P, N_C], f32, tag="ms")
                nc.sync.dma_start(ws[:], wT3[:, k, :])
                nc.sync.dma_start(ms[:], mT3[:, k, :])
                nc.vector.tensor_tensor(
                    wm[:, k, :], ws[:], ms[:], mybir.AluOpType.mult
                )

            # bias broadcast across partitions: [N_C] -> [P, N_C]
            bias_t = wres.tile([P, N_C], f32, tag="bias")
            nc.gpsimd.dma_start(bias_t[:], bias[None, :].broadcast_to([P, N_C]))

            # --- phase 2: matmul over m-tiles ---
            for g in range(M_C // MG):           # 32
                dg = dpool.tile([P, KS, MG], f32r, tag="dg")
                # SWDGE casting DMA: fp32 DRAM -> fp32r SBUF (rounds)
                nc.gpsimd.dma_start(dg[:], dT3[:, :, g * MG:(g + 1) * MG])
                for nt in range(N_C // N_TILE):  # 2
                    ps = pspool.tile([P, N_TILE], f32, tag="ps")
                    nsl = slice(nt * N_TILE, (nt + 1) * N_TILE)
                    for k in range(KS):
                        nc.tensor.matmul(
                            ps[:],
                            lhsT=dg[:, k, :],
                            rhs=wm[:, k, nsl],
                            start=(k == 0),
                            stop=(k == KS - 1),
                        )
                    ot = opool.tile([P, N_TILE], f32, tag="ot")
                    nc.vector.tensor_tensor(
                        ot[:], ps[:], bias_t[:, nsl], mybir.AluOpType.add
                    )
                    nc.sync.dma_start(out3[:, g, nsl], ot[:])

    nc.compile()
    return nc


def _get_nc():
    global _NC_CACHE
    if _NC_CACHE is None:
        _NC_CACHE = _build_nc()
    return _NC_CACHE


def prepare_in_maps(data, weight, mask, bias):
    """Host-side layout marshalling: K-major per-core shards."""
    data = np.ascontiguousarray(np.asarray(data), dtype=np.float32)
    weight = np.asarray(weight, dtype=np.float32)
    mask = np.asarray(mask, dtype=np.float32)
    bias = np.asarray(bias, dtype=np.float32)

    d_sh = [np.ascontiguousarray(data[i * M_C:(i + 1) * M_C].T)
            for i in range(M_SH)]                       # each [K, M_C]
    w_sh = [np.ascontiguousarray(weight[j * N_C:(j + 1) * N_C].T)
            for j in range(N_SH)]                       # each [K, N_C]
    m_sh = [np.ascontiguousarray(mask[j * N_C:(j + 1) * N_C].T)
            for j in range(N_SH)]
    b_sh = [np.ascontiguousarray(bias[j * N_C:(j + 1) * N_C])
            for j in range(N_SH)]

    in_maps = []
    for core in range(M_SH * N_SH):
        i, j = divmod(core, N_SH)
        in_maps.append({"dT": d_sh[i], "wT": w_sh[j], "mT": m_sh[j],
                        "bias": b_sh[j]})
    return in_maps


def kernel(data, weight, mask, bias):
    global LAST_RESULTS
    in_maps = prepare_in_maps(data, weight, mask, bias)
    nc = _get_nc()
    trace = bool(int(os.environ.get("KERNEL_TRACE", "0")))
    res = run_bass_kernel_spmd(
        nc, in_maps, core_ids=list(range(M_SH * N_SH)),
        trace=trace,
    )
    LAST_RESULTS = res

    out = np.empty((M, N), dtype=np.float32)
    for core in range(M_SH * N_SH):
        i, j = divmod(core, N_SH)
        out[i * M_C:(i + 1) * M_C, j * N_C:(j + 1) * N_C] = \
            res.results[core]["out"]
    return out


# revision 16
# speedup vs baseline: 1.0170x; 1.0170x over previous
"""Trainium2 Bass kernel for nn_CusparseDynamicLinear:
    out = data @ (mask * weight)^T + bias
  data  [8192, 4096] fp32
  weight[4096, 4096] fp32
  mask  [4096, 4096] fp32 (0/1)
  bias  [4096]       fp32
  out   [8192, 4096] fp32

Strategy: 8 NeuronCores, sharded 2-way on tokens (M) x 4-way on
out_features (N). Host does pure layout marshalling (K-major,
partition-major device layout); all FLOPs (mask multiply, matmul,
bias add) run on device.

Per core:
  dT [G=32, P=128, KS=32, MG=128] fp32  (data shard, device layout)
  wT/mT [P=128, KS=32, N_C=1024] fp32   (weight/mask shards)
  bias_c [1024]
  out_c [M_C=4096, N_C=1024]

Device program: mask*weight on DVE producing a resident fp32r tile
(K on partitions); data m-groups DMA-cast fp32->fp32r in flight
(SWDGE, fully contiguous per partition); fp32r matmuls accumulate
K=4096 in PSUM; DVE adds bias from PSUM; HWDGE stores out.
fp32r streams at bf16 speed for free-dim>=256 with ~16x better
accuracy (measured absmax rel err ~1.5e-4 vs 2.6e-3 for bf16).
"""
import os
import numpy as np

import concourse.bass as bass
import concourse.tile as tile
import concourse.mybir as mybir
from concourse import bacc
from concourse.bass_utils import run_bass_kernel_spmd

M, N, K = 8192, 4096, 4096
M_SH, N_SH = 2, 4
M_C, N_C = M // M_SH, N // N_SH   # 4096, 1024
P = 128
KS = K // P                       # 32 k-subtiles
N_TILE = 512                      # psum free dim (one fp32 bank)
MG = 128                          # m-tile (stationary free dim)
G = M_C // MG                     # 32 m-groups
W_CHUNK = 512                     # weight staging chunk along N

_NC_CACHE = None
LAST_RESULTS = None  # BassKernelResults of the most recent run (for test.py)


def _build_nc():
    nc = bacc.Bacc()
    f32 = mybir.dt.float32
    f32r = mybir.dt.float32r

    dT = nc.dram_tensor("dT", [G, P, KS, MG], f32, kind="ExternalInput")
    wT = nc.dram_tensor("wT", [P, KS, N_C], f32, kind="ExternalInput")
    mT = nc.dram_tensor("mT", [P, KS, N_C], mybir.dt.uint8,
                        kind="ExternalInput")
    bias = nc.dram_tensor("bias", [N_C], f32, kind="ExternalInput")
    out = nc.dram_tensor("out", [M_C, N_C], f32, kind="ExternalOutput")

    out3 = out.rearrange("(g p) n -> p g n", p=P)

    with tile.TileContext(nc) as tc:
        from concourse.tile_rust import add_dep_helper
        with (
            tc.tile_pool(name="wres", bufs=1) as wres,
            tc.tile_pool(name="wstage", bufs=2) as wstage,
            tc.tile_pool(name="dpool", bufs=4) as dpool,
            tc.tile_pool(name="opool", bufs=3) as opool,
            tc.tile_pool(name="pspool", bufs=8, space="PSUM") as pspool,
        ):
            wm = wres.tile([P, KS, N_C], f32r, tag="wm")
            tts = {}

            def w_strip(h, k):
                """Load one [P, N_TILE] weight+mask strip and produce the
                masked fp32r strip on DVE. Mask rides as uint8 (exact for
                0/1) to cut the startup weight traffic."""
                nsl = slice(h * N_TILE, (h + 1) * N_TILE)
                ws = wstage.tile([P, N_TILE], f32, tag="ws")
                ms = wstage.tile([P, N_TILE], mybir.dt.uint8, tag="ms")
                nc.sync.dma_start(ws[:], wT[:, k, nsl])
                nc.sync.dma_start(ms[:], mT[:, k, nsl])
                tt = nc.vector.tensor_tensor(
                    wm[:, k, nsl], ws[:], ms[:], mybir.AluOpType.mult
                )
                tts[(h, k)] = tt.ins

            # --- phase 1 (nt=0 half): the first psum groups can retire
            # after only this half of the masked weights is resident ---
            for k in range(KS):
                w_strip(0, k)
            for k in range(KS):
                w_strip(1, k)

            def load_dg(g, after=None):
                dg = dpool.tile([P, KS, MG], f32r, tag="dg")
                # SWDGE casting DMA: fp32 DRAM -> fp32r SBUF (rounds);
                # fully contiguous 16KB per partition
                dma = nc.gpsimd.dma_start(dg[:], dT[g])
                if after is not None:
                    # keep early dg prefetch from stealing DMA bandwidth
                    # from the weight strips that gate the first groups
                    add_dep_helper(dma.ins, after, sync=True,
                                   reason="pace dg behind weight strips")
                return dg

            def mm_group(g, dg, nt):
                ps = pspool.tile([P, N_TILE], mybir.dt.float32, tag="ps")
                nsl = slice(nt * N_TILE, (nt + 1) * N_TILE)
                for k in range(KS):
                    nc.tensor.matmul(
                        ps[:],
                        lhsT=dg[:, k, :],
                        rhs=wm[:, k, nsl],
                        start=(k == 0),
                        stop=(k == KS - 1),
                    )
                ot = opool.tile([P, N_TILE], mybir.dt.float32, tag="ot")
                nc.vector.tensor_tensor(
                    ot[:], ps[:], bias_t[:, nsl], mybir.AluOpType.add
                )
                nc.scalar.dma_start(out3[:, g, nsl], ot[:])

            # --- phase 2: the PE queue is strict FIFO, so no (g, nt=1)
            # group may sit in the queue before the nt=1 weight half is
            # resident -- it would head-of-line-block ready nt=0 work.
            # The first PRE groups therefore run nt=0 only; their nt=1
            # passes run at the very end with re-loaded dg tiles (16MB of
            # extra DMA on a pipe that is idle by then). ---
            PRE = 8
            # bias broadcast across partitions: [N_C] -> [P, N_C]
            bias_t = wres.tile([P, N_C], f32, tag="bias")
            nc.gpsimd.dma_start(
                bias_t[:], bias[None, :].broadcast_to([P, N_C]))
            for g in range(PRE):
                mm_group(g, load_dg(g), 0)
            for g in range(PRE, G):
                dg = load_dg(g)
                mm_group(g, dg, 0)
                mm_group(g, dg, 1)
            for g in range(PRE):
                mm_group(g, load_dg(g), 1)

    nc.compile()
    return nc


def _get_nc():
    global _NC_CACHE
    if _NC_CACHE is None:
        _NC_CACHE = _build_nc()
    return _NC_CACHE


def prepare_in_maps(data, weight, mask, bias):
    """Host-side layout marshalling: device-layout per-core shards."""
    data = np.ascontiguousarray(np.asarray(data), dtype=np.float32)
    weight = np.asarray(weight, dtype=np.float32)
    mask = np.asarray(mask, dtype=np.float32)
    bias = np.asarray(bias, dtype=np.float32)

    # data shard i: [M_C, K] -> [K, M_C] -> (KS, P, G, MG) -> [G, P, KS, MG]
    d_sh = []
    for i in range(M_SH):
        dT = data[i * M_C:(i + 1) * M_C].T            # [K, M_C]
        d4 = dT.reshape(KS, P, G, MG).transpose(2, 1, 0, 3)
        d_sh.append(np.ascontiguousarray(d4))

    # weight/mask shard j: [N_C, K] -> [K, N_C] -> (KS, P, N_C) -> [P, KS, N_C]
    def _wprep(a, j):
        aT = a[j * N_C:(j + 1) * N_C].T               # [K, N_C]
        return np.ascontiguousarray(
            aT.reshape(KS, P, N_C).transpose(1, 0, 2))

    w_sh = [_wprep(weight, j) for j in range(N_SH)]
    # mask is exactly 0.0/1.0; ship as uint8 (lossless, 4x less traffic)
    m_sh = [_wprep(mask, j).astype(np.uint8) for j in range(N_SH)]
    b_sh = [np.ascontiguousarray(bias[j * N_C:(j + 1) * N_C])
            for j in range(N_SH)]

    in_maps = []
    for core in range(M_SH * N_SH):
        i, j = divmod(core, N_SH)
        in_maps.append({"dT": d_sh[i], "wT": w_sh[j], "mT": m_sh[j],
                        "bias": b_sh[j]})
    return in_maps


def kernel(data, weight, mask, bias):
    global LAST_RESULTS
    in_maps = prepare_in_maps(data, weight, mask, bias)
    nc = _get_nc()
    trace = bool(int(os.environ.get("KERNEL_TRACE", "0")))
    res = run_bass_kernel_spmd(
        nc, in_maps, core_ids=list(range(M_SH * N_SH)),
        trace=trace,
    )
    LAST_RESULTS = res

    out = np.empty((M, N), dtype=np.float32)
    for core in range(M_SH * N_SH):
        i, j = divmod(core, N_SH)
        out[i * M_C:(i + 1) * M_C, j * N_C:(j + 1) * N_C] = \
            res.results[core]["out"]
    return out
